# revision 46
# baseline (speedup 1.0000x reference)
"""Trainium2 Bass kernel for nn_CrossAttention (B=4, T=1024, S=2048, D=1024, H=16).

Sharding: tensor-parallel over heads. Each of the 8 cores owns 2 heads
(a 128-column slice of the q/k/v projections and the matching 128-row
slice of the o-projection input). Every core computes a full-shape
partial o-proj output; the host sums the 8 partials (the all-reduce is
done on the host during the gather/unshard step).

Layout strategy: all device matmuls contract along the SBUF partition
axis, so the host pre-transposes the activations and weights (free on
host, removes every on-chip transpose):
  xT  [D, B*T]  = query^T          (bf16)
  kvT [D, B*S]  = key_value^T      (bf16)
  wqT/wkT/wvT [D, 128] per core    (bf16)
  woT [128, D] per core            (bf16)

Pipeline per core (all matmul accumulation in fp32 PSUM):
  qT = WqT.T @ xT            -> [128c, B*T]   (c on partitions)
  kT = WkT.T @ kvT           -> [128c, B*S]
  V  = kvT.T @ WvT           -> [s, c] tiles, stored ones-augmented [128, 65]
  scoresT = kT.T @ qT per head (K=64)         -> [128s, 512t] PSUM
  PT = exp(0.125 * scoresT)  (ACT, no max-subtraction: |scores| < ~7)
  attnT[h] += V_aug.T @ PT   -> [65, 512t]; row 64 = softmax rowsum (free)
Epilogue (per 512-t block, normalization BEFORE o-proj so both heads
merge into one K=128 o-proj):
  rinv[1,512]  = approx-recip(attnT[64,:])       (DVE, fp16)
  rb[64,512]   = ones[1,64].T @ rinv             (PE broadcast, K=1)
  ATn[128,512] = attnT[0:64] * rb   per head     (DVE, bf16, heads stacked
                                                  on partitions)
  o-proj: out[128t, 512d] = ATn[:,tsub].T @ woT  (K=128, PSUM -> DRAM DMA
          directly); deferred into the next block's j-iters as PE filler.
"""

import os
import sys
from contextlib import ExitStack

import numpy as np

for _p in (
    "/root/.axon_site",
    "/root/.axon_site/_ro/trn_rl_repo",
    "/root/.axon_site/_ro/pypackages",
    "/opt/trn_rl_repo",
):
    if os.path.isdir(_p) and _p not in sys.path:
        sys.path.append(_p)

import ml_dtypes  # noqa: E402

import concourse.bass as bass  # noqa: E402
import concourse.mybir as mybir  # noqa: E402
import concourse.tile as tile  # noqa: E402
from concourse import bacc  # noqa: E402

BF = mybir.dt.bfloat16
F16 = mybir.dt.float16
F32 = mybir.dt.float32
NPBF = ml_dtypes.bfloat16

B, T, S, D = 4, 1024, 2048, 1024
BT, BS = B * T, B * S
P = 128
NCORES = 8
KT = D // P          # 8 contraction tiles of 128
TCH = 512            # free-dim chunk for projections / attention t-chunks
NJ = S // P          # 16 s-tiles of 128 per batch
NST = BS // P        # 64 s-tiles total
EXP_SCALE = float(64 ** -0.5)  # folded into the ACT exp
DEBUG_EPI = False


def build_nc():
    nc = bacc.Bacc("TRN2", target_bir_lowering=False)

    xT = nc.dram_tensor("xT", [D, BT], BF, kind="ExternalInput").ap()
    kvT = nc.dram_tensor("kvT", [D, BS], BF, kind="ExternalInput").ap()
    wqT = nc.dram_tensor("wqT", [D, P], BF, kind="ExternalInput").ap()
    wkT = nc.dram_tensor("wkT", [D, P], BF, kind="ExternalInput").ap()
    wvT = nc.dram_tensor("wvT", [D, P], BF, kind="ExternalInput").ap()
    woT = nc.dram_tensor("woT", [P, D], BF, kind="ExternalInput").ap()
    out = nc.dram_tensor("out", [BT, D], F32, kind="ExternalOutput").ap()
    if DEBUG_EPI:
        dbg_r = nc.dram_tensor("dbg_r", [1, 2, TCH], F32, kind="ExternalOutput").ap()
        dbg_rinv32 = nc.dram_tensor("dbg_rinv32", [1, 2, TCH], F32, kind="ExternalOutput").ap()
        dbg_rinv = nc.dram_tensor("dbg_rinv", [1, 2, TCH], F16, kind="ExternalOutput").ap()
        dbg_atn = nc.dram_tensor("dbg_atn", [P, TCH], BF, kind="ExternalOutput").ap()

    with tile.TileContext(nc) as tc, ExitStack() as ctx:
        consts = ctx.enter_context(tc.tile_pool(name="consts", bufs=1))
        big = ctx.enter_context(tc.tile_pool(name="big", bufs=1))
        xin = ctx.enter_context(tc.tile_pool(name="xin", bufs=2))
        ptp = ctx.enter_context(tc.tile_pool(name="ptp", bufs=4))
        smalls = ctx.enter_context(tc.tile_pool(name="smalls", bufs=4))
        atn_pool = ctx.enter_context(tc.tile_pool(name="atn", bufs=2))
        atsb_pool = ctx.enter_context(tc.tile_pool(name="atsb", bufs=2))
        ost_pool = ctx.enter_context(tc.tile_pool(name="ost", bufs=3))
        # PSUM budget (8 banks): mm 2x[128,1024]=4 + at 2 tags x[65,512]=2
        #                        + op 1x[128,512]=1 + ops 2x[128,512]=2  -> 9?
        # no: at tags are bufs=1 each (ats lifetime ends inside the block,
        # epilogue is inline), so 4 + 2 + 1 + 2 would be 9 — run ops at
        # bufs=2 and op at bufs=1 sharing... total must be <= 8:
        # mm(4) + at(2) + op(1) + ops... ops gets bufs=1 mid-stream; the
        # ping-pong loss is hidden by filler slack.
        mm_ps = ctx.enter_context(tc.tile_pool(name="mm_ps", bufs=2, space="PSUM"))
        at_pool = ctx.enter_context(tc.tile_pool(name="at_ps", bufs=1, space="PSUM"))
        op_pool = ctx.enter_context(tc.tile_pool(name="op_ps", bufs=2, space="PSUM"))
        ops_pool = op_pool

        # ---- warmup constants (no DMA dependency) ----
        warm_t = consts.tile([1, TCH], BF, tag="warm_t")
        nc.vector.memset(warm_t[:], 1.0)
        ones64 = consts.tile([1, 64], F16, tag="ones64")
        nc.vector.memset(ones64[:], 1.0)

        # ---- resident weights ----
        # DMA issue order = per-queue FIFO priority: wq first (q-proj starts
        # earliest), x0 is issued right after in the prologue below, then
        # wk/wv/wo, then kv0 chunk-major.
        wq_s = consts.tile([P, KT, P], BF, tag="wq_s")
        wk_s = consts.tile([P, KT, P], BF, tag="wk_s")
        wv_s = consts.tile([P, KT, P], BF, tag="wv_s")
        wqT_t = wqT.rearrange("(kt p) c -> p kt c", p=P)
        wkT_t = wkT.rearrange("(kt p) c -> p kt c", p=P)
        wvT_t = wvT.rearrange("(kt p) c -> p kt c", p=P)
        for kt in range(KT):
            nc.sync.dma_start(wq_s[:, kt, :], wqT_t[:, kt, :])
        wo_s = consts.tile([P, D], BF, tag="wo_s")

        # ---- resident intermediates ----
        qT_s = big.tile([P, BT], BF, tag="qT_s")
        kT_s = big.tile([P, BS], BF, tag="kT_s")
        # Per-head V, ones-augmented + padded to 80 cols (aligned weight
        # loads): 64 s-tiles, each [128, 80] with col 64 == 1.0
        v_s = [
            big.tile([P, NST, 80], BF, tag=f"v{h}_s", name=f"v{h}_s")
            for h in range(2)
        ]
        for h in range(2):
            nc.gpsimd.memset(v_s[h][:], 1.0)

        xT_t = xT.rearrange("(kt p) t -> p kt t", p=P)
        kvT_t = kvT.rearrange("(kt p) s -> p kt s", p=P)

        def warm(iters, tag):
            # p-state keep-alive matmuls on a memset tile (no DMA deps)
            warm_ps = op_pool.tile([P, TCH], F32, tag="op", name=f"warm_{tag}")
            for _ in range(iters):
                nc.tensor.matmul(
                    warm_ps[0:1, :], warm_t[0:1, 0:1], warm_t[:],
                    start=True, stop=True,
                )

        warm_sb = smalls.tile([1, 1], F32, tag="wsb", name="warm_sb")
        nc.scalar.activation(
            warm_sb[:], warm_t[0:1, 0:1], mybir.ActivationFunctionType.Exp
        )

        def load_x_batch(b):
            # whole batch: per-partition runs of 2KB -> good DMA descriptors
            x_t = xin.tile([P, KT, T], BF, tag="x_t", name="x_t")
            for kt in range(KT):
                nc.sync.dma_start(x_t[:, kt, :], xT_t[:, kt, b * T:(b + 1) * T])
            return x_t

        def load_kv_batch(b, chunk_order=False):
            kv_t = xin.tile([P, KT, S], BF, tag="kv_t", name="kv_t")
            if chunk_order:
                # chunk-major: s-chunk 0 of every k-tile lands first so the
                # first attention j-iters can start before the whole batch
                # arrives
                for q in range(4):
                    for kt in range(KT):
                        nc.sync.dma_start(
                            kv_t[:, kt, q * TCH:(q + 1) * TCH],
                            kvT_t[:, kt, b * S + q * TCH:b * S + (q + 1) * TCH],
                        )
            else:
                for kt in range(KT):
                    nc.sync.dma_start(
                        kv_t[:, kt, :], kvT_t[:, kt, b * S:(b + 1) * S]
                    )
            return kv_t

        def q_proj_frags(x_t, b, half):
            # q projection for one 512-wide chunk, split into 2 fragments
            # (4 k-tiles each) so it can be spread across attention j-iters
            ch = 2 * b + half
            state = {}

            def frag(kts):
                def run():
                    if "ps" not in state:
                        state["ps"] = op_pool.tile([P, TCH], F32, tag="op", name="qps")
                    ps = state["ps"]
                    for kt in kts:
                        nc.tensor.matmul(
                            ps[:], wq_s[:, kt, :],
                            x_t[:, kt, half * TCH:(half + 1) * TCH],
                            start=(kt == 0), stop=(kt == KT - 1),
                        )
                    if KT - 1 in kts:
                        nc.vector.tensor_copy(
                            qT_s[:, ch * TCH:(ch + 1) * TCH], ps[:]
                        )
                return run

            return [frag(range(0, 4)), frag(range(4, KT))]

        def kv_proj_frags(kv_t, b, quarter):
            # kT projection (1 fragment) + V projection (2 fragments) for one
            # 512-wide kv chunk
            ch = 4 * b + quarter
            q0 = quarter * TCH
            state = {}

            def k_frag():
                ps = op_pool.tile([P, TCH], F32, tag="op", name="kps")
                for kt in range(KT):
                    nc.tensor.matmul(
                        ps[:], wk_s[:, kt, :], kv_t[:, kt, q0:q0 + TCH],
                        start=(kt == 0), stop=(kt == KT - 1),
                    )
                nc.vector.tensor_copy(kT_s[:, ch * TCH:(ch + 1) * TCH], ps[:])

            # V projection: [s, c] orientation, 4 s-subtiles share one bank.
            def v_frag(kts):
                def run():
                    if "vps" not in state:
                        state["vps"] = op_pool.tile(
                            [P, 4, P], F32, tag="op", name="vps"
                        )
                    vps = state["vps"]
                    for kt in kts:
                        for sub in range(4):
                            nc.tensor.matmul(
                                vps[:, sub, :],
                                kv_t[:, kt, q0 + sub * P:q0 + (sub + 1) * P],
                                wv_s[:, kt, :],
                                start=(kt == 0 and sub == 0),
                                stop=(kt == KT - 1 and sub == 3),
                            )
                    if KT - 1 in kts:
                        for sub in range(4):
                            jg = ch * 4 + sub
                            nc.vector.tensor_copy(
                                v_s[0][:, jg, 0:64], vps[:, sub, 0:64]
                            )
                            nc.vector.tensor_copy(
                                v_s[1][:, jg, 0:64], vps[:, sub, 64:128]
                            )
                return run

            return [k_frag, v_frag(range(0, 4)), v_frag(range(4, KT))]

        def attention_block(b, t2, fillers=()):
            # fillers: [(j, fn)] — PE filler work (next batch's projections,
            # previous block's o-proj) issued after iteration j so the tensor
            # engine never idles long enough to drop out of the top p-state.
            fmap = {}
            for j, fn in fillers:
                fmap.setdefault(j, []).append(fn)
            t0 = b * T + t2 * TCH
            ats = [
                at_pool.tile([65, TCH], F32, tag=f"at{h}", name=f"at{h}")
                for h in range(2)
            ]
            pts = {}

            def attnv(j):
                jg = b * NJ + j
                pt = pts.pop(j)
                for h in range(2):
                    nc.tensor.matmul(
                        ats[h][:],
                        v_s[h][:, jg, 0:65],
                        pt[:, h * TCH:(h + 1) * TCH],
                        start=(j == 0), stop=(j == NJ - 1),
                    )

            # attnV runs one j behind scores/exp: the ~1.1us exp latency is
            # covered by a full slot of other PE work instead of a PE park
            for j in range(NJ):
                sc = mm_ps.tile([P, 1024], F32, tag="mm", name="sc")
                for h in range(2):
                    hp = h * 64
                    nc.tensor.matmul(
                        sc[:, h * TCH:(h + 1) * TCH],
                        kT_s[hp:hp + 64, b * S + j * P: b * S + (j + 1) * P],
                        qT_s[hp:hp + 64, t0:t0 + TCH],
                        start=True, stop=True,
                    )
                pt = ptp.tile([P, 1024], BF, tag="pt", name="pt")
                nc.scalar.activation(
                    pt[:], sc[:],
                    mybir.ActivationFunctionType.Exp,
                    scale=EXP_SCALE,
                )
                pts[j] = pt
                if j > 1:
                    attnv(j - 2)
                for fn in fmap.get(j, ()):
                    fn()
            attnv(NJ - 2)
            attnv(NJ - 1)

            # ---- epilogue ----
            # 1) immediately copy A^T and the rowsum row out of PSUM (split
            #    across ACT and DVE) so the next block's attnV can reuse the
            #    ats banks after ~1us instead of waiting the whole chain
            aT_sb = atsb_pool.tile([P, TCH], BF, tag="atsb", name="aT_sb")
            r_sb = smalls.tile([1, 2, TCH], F32, tag="r_sb", name="r_sb", bufs=2)
            nc.vector.tensor_copy(aT_sb[0:64, :], ats[0][0:64, :])
            nc.vector.tensor_copy(aT_sb[64:128, :], ats[1][0:64, :])
            nc.vector.tensor_copy(r_sb[0:1, 0, :], ats[0][64:65, :])
            nc.vector.tensor_copy(r_sb[0:1, 1, :], ats[1][64:65, :])
            # 2) lazy chain overlapped with the next block's attention:
            #    rinv = 1/r (DVE approx, ~51 ULP) -> fp16 cast (ACT) ->
            #    K=1 ones-matmul broadcast (PE) -> normalize (DVE)
            rinv32 = smalls.tile([1, 2, TCH], F32, tag="rinv32", name="rinv32", bufs=2)
            nc.vector.reciprocal_approx_fast(rinv32[:], r_sb[:])
            rinv = smalls.tile([1, 2, TCH], F16, tag="rinv", name="rinv", bufs=2)
            nc.vector.tensor_copy(rinv[:], rinv32[:])
            rb = op_pool.tile([P, TCH], F32, tag="op", name="rb")
            for h in range(2):
                nc.tensor.matmul(
                    rb[h * 64:(h + 1) * 64, :], ones64[:], rinv[0:1, h, :],
                    start=True, stop=True,
                )
            # ATn[c, t] = A^T * (1/r): SBUF aT x PSUM rb, same partition base
            atn = atn_pool.tile([P, TCH], BF, tag="atn", name="atn")
            for h in range(2):
                nc.vector.tensor_tensor(
                    atn[h * 64:(h + 1) * 64, :],
                    aT_sb[h * 64:(h + 1) * 64, :],
                    rb[h * 64:(h + 1) * 64, :],
                    mybir.AluOpType.mult,
                )

            # o-proj fillers for the NEXT block: K=128 (heads merged).
            # DMA cannot read PSUM, so stage through SBUF; the psum->sbuf
            # copies mostly ride DVE, some ACT (which idles at the tail).
            def oproj(sub, n, use_act):
                def run():
                    ops = ops_pool.tile([P, TCH], F32, tag="op", name="ops")
                    nc.tensor.matmul(
                        ops[:],
                        atn[:, sub * P:(sub + 1) * P],
                        wo_s[:, n * TCH:(n + 1) * TCH],
                        start=True, stop=True,
                    )
                    ost = ost_pool.tile([P, TCH], F32, tag="ost", name="ost")
                    if use_act:
                        nc.scalar.copy(ost[:], ops[:])
                    else:
                        nc.vector.tensor_copy(ost[:], ops[:])
                    nc.sync.dma_start(
                        out[t0 + sub * P:t0 + (sub + 1) * P,
                            n * TCH:(n + 1) * TCH],
                        ost[:],
                    )
                return run

            last = (b == B - 1) and (t2 == 1)
            if DEBUG_EPI and last:
                nc.sync.dma_start(dbg_r, r_sb[:])
                nc.sync.dma_start(dbg_rinv32, rinv32[:])
                nc.sync.dma_start(dbg_rinv, rinv[:])
                nc.sync.dma_start(dbg_atn, atn[:])
            # mid-stream: 6 copies on DVE, 2 on ACT (ACT is exp-bound);
            # tail (last block): alternate so both engines drain in parallel
            fns = [
                oproj(sub, n, use_act=((sub * 2 + n) % 2 == 0) if last
                      else False)
                for sub in range(4) for n in range(2)
            ]
            if last:
                for fn in fns:
                    fn()
                return []
            return fns

        # b-major pipeline: batch 0 projects q + kv-chunk0 upfront (warm
        # matmuls cover the DMA lead-in), kv chunks 1-3 project as fillers
        # inside block (0,0); batch b+1's projections are interleaved into
        # batch b's attention; block i's o-proj into block i+1's attention.
        x_t = load_x_batch(0)
        for kt in range(KT):
            nc.sync.dma_start(wk_s[:, kt, :], wkT_t[:, kt, :])
            nc.sync.dma_start(wv_s[:, kt, :], wvT_t[:, kt, :])
        nc.sync.dma_start(wo_s[:], woT)
        kv_t = load_kv_batch(0, chunk_order=True)
        warm(22, "a")  # ~6.5us: wq + x0 in flight
        for fn in q_proj_frags(x_t, 0, 0) + q_proj_frags(x_t, 0, 1):
            fn()
        warm(14, "b")  # ~3us: kv0 chunk 0 in flight
        for fn in kv_proj_frags(kv_t, 0, 0):
            fn()
        # kv chunks 1..3 of batch 0: fillers in block (0,0), just ahead of
        # the j-iters that consume them (chunk c feeds j in [4c, 4c+4))
        f_kv0 = []
        kv0_slots = {1: [0, 1, 2], 2: [3, 4, 5], 3: [6, 7, 8]}
        for quarter in range(1, 4):
            f_kv0 += list(zip(kv0_slots[quarter],
                              kv_proj_frags(kv_t, 0, quarter)))
        # Filler slot maps. Two rules: every j-slot needs >= ~0.5us of
        # extra PE work (or the engine parks on the exp latency, idles, and
        # drops out of the top p-state: any gap costs ~3us of half-clock
        # ramp), and the shared 2-slot psum ring must see chain allocations
        # contiguously (a chain holding a slot across many j-slots while
        # others allocate overflows the 4-deep dependency wait queue).
        op_fill = f_kv0  # consumed below for block (0,0)
        extra_next0 = []
        for b in range(B):
            if b + 1 < B:
                nx_t = load_x_batch(b + 1)
                nkv_t = load_kv_batch(b + 1)
                frags0 = (
                    q_proj_frags(nx_t, b + 1, 0)
                    + q_proj_frags(nx_t, b + 1, 1)
                    + kv_proj_frags(nkv_t, b + 1, 0)
                )
                if b == 0:
                    # block (0,0) also projects batch 0's kv chunks 1-3
                    f0 = op_fill + list(zip(
                        [9, 10, 11, 12, 13, 14, 15], frags0))
                else:
                    f0 = list(zip([1, 2, 3, 4, 5, 6, 7], frags0))
                    f0 += list(zip([8, 9, 10, 11, 12, 13, 14, 15], op_fill))
                if b + 1 == B - 1:
                    # last batch: defer kv chunks 2,3 into block (B-1,0) so
                    # the late blocks keep their PE filler
                    f1 = list(zip([0, 1, 2], kv_proj_frags(nkv_t, b + 1, 1)))
                    extra = (
                        kv_proj_frags(nkv_t, b + 1, 2)
                        + kv_proj_frags(nkv_t, b + 1, 3)
                    )
                    extra_next0 = list(zip([0, 1, 2, 3, 4, 5], extra))
                else:
                    frags1 = (
                        kv_proj_frags(nkv_t, b + 1, 1)
                        + kv_proj_frags(nkv_t, b + 1, 2)
                        + kv_proj_frags(nkv_t, b + 1, 3)
                    )
                    f1 = list(zip([0, 1, 2, 3, 4, 5, 6, 7, 8], frags1))
            else:
                # final batch: deferred kv chunks 2,3 then (2,1)'s o-proj
                f0 = extra_next0 + list(
                    zip([6, 7, 8, 9, 10, 11, 12, 13], op_fill))
                extra_next0 = []
                f1 = []
            ofill0 = attention_block(b, 0, f0)
            # o-proj of (b,0) rides (b,1) at slots 3.. interleaved with f1
            if b + 1 < B and b + 1 == B - 1:
                f1 += list(zip([3, 4, 5, 6, 8, 9, 10, 11], ofill0))
            elif b + 1 < B:
                f1 += list(zip([9, 10, 11, 12, 12, 13, 14, 15], ofill0))
            else:
                f1 += list(zip([8, 9, 10, 11, 12, 13, 14, 15], ofill0))
            op_fill = attention_block(b, 1, f1)

    nc.compile()
    return nc


_NC_CACHE = None


def _get_nc():
    global _NC_CACHE
    if _NC_CACHE is None:
        _NC_CACHE = build_nc()
    return _NC_CACHE


def make_in_maps(query, key_value, wq, wk, wv, wo):
    q2 = np.ascontiguousarray(np.asarray(query, np.float32).reshape(BT, D))
    kv2 = np.ascontiguousarray(np.asarray(key_value, np.float32).reshape(BS, D))
    xT = np.ascontiguousarray(q2.astype(NPBF).T)
    kvT = np.ascontiguousarray(kv2.astype(NPBF).T)
    wq = np.asarray(wq, np.float32)
    wk = np.asarray(wk, np.float32)
    wv = np.asarray(wv, np.float32)
    wo = np.asarray(wo, np.float32)
    in_maps = []
    for c in range(NCORES):
        cs = slice(c * P, (c + 1) * P)
        in_maps.append({
            "xT": xT,
            "kvT": kvT,
            "wqT": np.ascontiguousarray(wq[cs, :].astype(NPBF).T),
            "wkT": np.ascontiguousarray(wk[cs, :].astype(NPBF).T),
            "wvT": np.ascontiguousarray(wv[cs, :].astype(NPBF).T),
            "woT": np.ascontiguousarray(wo[:, cs].astype(NPBF).T),
        })
    return in_maps


def run(inputs, trace=False, **kwargs):
    from concourse.bass_utils import run_bass_kernel_spmd

    nc = _get_nc()
    in_maps = make_in_maps(**inputs)
    res = run_bass_kernel_spmd(
        nc, in_maps, core_ids=list(range(NCORES)), trace=trace, **kwargs
    )
    acc = np.zeros((BT, D), np.float64)
    for r in res.results:
        acc += r["out"].astype(np.float64)
    return acc.astype(np.float32).reshape(B, T, D), res


def kernel(**inputs):
    return run(inputs, trace=False)[0]
